# revision 14
# baseline (speedup 1.0000x reference)
"""DeepGCN (gnn_message_passing) Trainium2 Bass kernel, 8-way node-sharded SPMD.

Strategy (per core, nodes sharded 8 ways):
- Activations kept transposed hT [128 feats, RPAD rows] in SBUF.
- Dense y = h@W + b: PE matmuls lhsT=hT-tile rhs=W (+rank-1 ones-matmul bias)
  -> row-major y tiles -> DMA to DRAM [HALF, 2F] (two column halves) ->
  ONE AllGather per layer -> table [TBL=HALF*8, 2F] (Shared).  Splitting each
  rank's rows into two column-halves keeps every gather index < 32768 (int16).
- spmm out[r] = sum_e val[e] * y[col[e]]: dma_gather 128 edges/tile into
  partitions, selector SEL[e,r] = (iota==rowrel[e])*val[e] built in ONE fused
  DVE scalar_tensor_tensor, PE matmul g.T @ SEL accumulated in PSUM per
  128-row block -> transposed result updates hT directly (relu/residual fused).
- Final spmm uses lhsT=SEL, rhs=g -> row-major [rows, C] -> output shard.

Edges are preprocessed on host (numpy): sorted by destination row-block, split
per block into the two table halves, padded to a fixed number of 128-edge
tiles per (block, half) so one static program serves all 8 cores.
"""

import numpy as np

import concourse.bacc as bacc
import concourse.bass as bass
import concourse.mybir as mybir
import concourse.tile as tile
from concourse import library_config
from concourse.bass_utils import run_bass_kernel_spmd

NCORES = 8
P = 128


class Cfg:
    def __init__(self, N=40000, E=640000, DIN=256, H=128, C=64, L=2, SBB=5,
                 tbl_fp16=True):
        assert N % NCORES == 0
        self.N, self.E, self.DIN, self.H, self.C, self.L = N, E, DIN, H, C, L
        self.NSH = N // NCORES                    # rows per core
        self.NBLK = -(-self.NSH // P)             # 128-row blocks per core
        self.RPAD = self.NBLK * P
        assert self.NSH % 2 == 0
        self.HALF = self.NSH // 2                 # rows per table half per core
        self.TBL = self.HALF * NCORES             # rows per gather table
        assert self.TBL < 32768, "gather indices must fit int16"
        assert self.NBLK % SBB == 0
        self.SBB = SBB                            # blocks per superblock
        self.NSB = self.NBLK // SBB
        self.tbl_fp16 = tbl_fp16
        self.tdt = mybir.dt.float16 if tbl_fp16 else mybir.dt.float32
        self.tnp = np.float16 if tbl_fp16 else np.float32


CFG_FULL = Cfg()


# ---------------------------------------------------------------- host side


def _pack_idx(idx_flat):
    """[n] int16 -> [128, n//16]: slot i -> partition i%16, col i//16, x8 replicated."""
    n = idx_flat.shape[-1]
    t = idx_flat.reshape(*idx_flat.shape[:-1], n // 16, 16)
    t = np.swapaxes(t, -1, -2)                    # [..., 16, n//16]
    return np.tile(t, (1,) * (t.ndim - 2) + (8, 1)).astype(np.int16)


def _pack_pt(a_flat):
    """[n] -> [128, n//128]: slot i -> [i%128, i//128]."""
    n = a_flat.shape[-1]
    t = a_flat.reshape(*a_flat.shape[:-1], n // 128, 128)
    return np.swapaxes(t, -1, -2).copy()


def preprocess(cfg, x, edge_row, edge_col, edge_val):
    """Shard x, build per-core gather/selector metadata. Returns (per_core, TPB)."""
    er = np.asarray(edge_row).astype(np.int64)
    ec = np.asarray(edge_col).astype(np.int64)
    ev = np.asarray(edge_val).astype(np.float32)

    owner = er // cfg.NSH
    row_loc = er % cfg.NSH
    blk = row_loc // P                            # block within core
    rel = (row_loc % P).astype(np.float32)
    c_owner = ec // cfg.NSH
    c_loc = ec % cfg.NSH
    half = (c_loc >= cfg.HALF).astype(np.int64)
    tbl_idx = (c_owner * cfg.HALF + c_loc - half * cfg.HALF).astype(np.int64)

    cores = []
    max_cnt = 0
    for r in range(NCORES):
        m = owner == r
        cores.append((blk[m], half[m], tbl_idx[m], ev[m], rel[m]))
        key = blk[m] * 2 + half[m]
        cnt = np.bincount(key, minlength=cfg.NBLK * 2)
        max_cnt = max(max_cnt, int(cnt.max()))
    TPB = max(1, -(-max_cnt // P))                # tiles per (block, half)
    NIDX = cfg.SBB * TPB * P                      # gather-call size

    per_core = []
    for r in range(NCORES):
        b, h, ti, v, rl = cores[r]
        key = b * 2 + h
        # secondary sort by table index: monotone gather addresses within each
        # (block, half) group give far better HBM row locality
        order = np.argsort(key * 32768 + ti, kind="stable")
        b, h, ti, v, rl = b[order], h[order], ti[order], v[order], rl[order]
        cnt = np.bincount(key[order], minlength=cfg.NBLK * 2)
        # slot of edge j within its (b,h) group
        within = np.arange(len(b)) - np.repeat(
            np.concatenate([[0], np.cumsum(cnt)[:-1]]), cnt)
        # flat slot in [h, s, NIDX] layout
        s = b // cfg.SBB
        bb = b % cfg.SBB
        slot = bb * TPB * P + within
        idx_arr = np.zeros((2, cfg.NSB, NIDX), np.int16)
        val_arr = np.zeros((2, cfg.NSB, NIDX), np.float32)
        row_arr = np.zeros((2, cfg.NSB, NIDX), np.float32)
        idx_arr[h, s, slot] = ti.astype(np.int16)
        val_arr[h, s, slot] = v
        row_arr[h, s, slot] = rl

        xT = np.zeros((cfg.DIN, cfg.RPAD), np.float32)
        xT[:, : cfg.NSH] = np.asarray(x[r * cfg.NSH:(r + 1) * cfg.NSH]).T
        per_core.append(dict(
            xT=np.ascontiguousarray(xT),
            idx=_pack_idx(idx_arr),                       # [2,NSB,128,NIDX//16]
            val=_pack_pt(val_arr).astype(cfg.tnp),        # [2,NSB,128,SBB*TPB]
            rowrel=_pack_pt(row_arr).astype(cfg.tnp),
            val32=_pack_pt(val_arr),
            rowrel32=_pack_pt(row_arr),
        ))
    return per_core, TPB


# -------------------------------------------------------------- device side


def build_program(cfg, TPB, dt_val, no_cc=False):
    H, C, DIN, L = cfg.H, cfg.C, cfg.DIN, cfg.L
    NIDX = cfg.SBB * TPB * P
    NPT = cfg.SBB * TPB
    fdims = [H] * (L + 1) + [C]                   # gather-table feature dims
    tdts = [cfg.tdt] * (L + 1) + [mybir.dt.float32]   # final table f32 (256B rows)

    nc = bacc.Bacc("TRN2", target_bir_lowering=False, debug=False,
                   num_devices=NCORES)
    f32 = mybir.dt.float32

    xT_d = nc.dram_tensor("xT", [DIN, cfg.RPAD], f32, kind="ExternalInput")
    w1_d = nc.dram_tensor("w1", [DIN, H], f32, kind="ExternalInput")
    b1_d = nc.dram_tensor("b1", [1, H], f32, kind="ExternalInput")
    wm_d = nc.dram_tensor("wm", [L, H, H], f32, kind="ExternalInput")
    bm_d = nc.dram_tensor("bm", [L, 1, H], f32, kind="ExternalInput")
    w2_d = nc.dram_tensor("w2", [H, C], f32, kind="ExternalInput")
    b2_d = nc.dram_tensor("b2", [1, C], f32, kind="ExternalInput")
    iota_d = nc.dram_tensor("iota", [P, P], cfg.tdt, kind="ExternalInput")
    iota32_d = nc.dram_tensor("iota32", [P, P], f32, kind="ExternalInput")
    idx_d = nc.dram_tensor("idx", [2, cfg.NSB, P, NIDX // 16], mybir.dt.int16,
                           kind="ExternalInput")
    val_d = nc.dram_tensor("val", [2, cfg.NSB, P, NPT], cfg.tdt,
                           kind="ExternalInput")
    row_d = nc.dram_tensor("rowrel", [2, cfg.NSB, P, NPT], cfg.tdt,
                           kind="ExternalInput")
    val32_d = nc.dram_tensor("val32", [2, cfg.NSB, P, NPT], f32,
                             kind="ExternalInput")
    row32_d = nc.dram_tensor("rowrel32", [2, cfg.NSB, P, NPT], f32,
                             kind="ExternalInput")
    out_d = nc.dram_tensor("out", [cfg.NSH, C], f32, kind="ExternalOutput")

    ag_in = [nc.dram_tensor(f"ag_in{l}", [cfg.HALF, 2 * fdims[l]], tdts[l])
             for l in range(L + 2)]
    tables = [nc.dram_tensor(f"table{l}", [cfg.TBL, 2 * fdims[l]], tdts[l],
                             addr_space="Shared")
              for l in range(L + 2)]

    with tile.TileContext(nc) as tc:
        import contextlib
        with contextlib.ExitStack() as ctx:
            const = ctx.enter_context(tc.tile_pool(name="const", bufs=1))
            htp = ctx.enter_context(tc.tile_pool(name="ht", bufs=1))
            psum = ctx.enter_context(tc.tile_pool(name="psum", bufs=4, space="PSUM"))
            meta = ctx.enter_context(tc.tile_pool(name="meta", bufs=2))
            gpool = ctx.enter_context(tc.tile_pool(name="g", bufs=2))
            selp = ctx.enter_context(tc.tile_pool(name="sel", bufs=6))
            yp = ctx.enter_context(tc.tile_pool(name="y", bufs=3))

            nc.gpsimd.load_library(library_config.mlp)

            # ---- constants
            nkt = DIN // P                       # k-tiles for layer-1 dense
            w1_sb = [const.tile([P, H], f32, name=f"w1sb{k}")
                     for k in range(nkt)]
            for k in range(nkt):
                nc.sync.dma_start(w1_sb[k][:], w1_d[k * P:(k + 1) * P, :])
            b1_sb = const.tile([1, H], f32)
            nc.sync.dma_start(b1_sb[:], b1_d[:])
            wm_sb = [const.tile([P, H], f32, name=f"wmsb{i}")
                     for i in range(L)]
            bm_sb = [const.tile([1, H], f32, name=f"bmsb{i}")
                     for i in range(L)]
            for i in range(L):
                nc.sync.dma_start(wm_sb[i][:], wm_d[i])
                nc.sync.dma_start(bm_sb[i][:], bm_d[i])
            w2_sb = const.tile([P, C], f32)
            nc.sync.dma_start(w2_sb[:], w2_d[:])
            b2_sb = const.tile([1, C], f32)
            nc.sync.dma_start(b2_sb[:], b2_d[:])
            iota_sb = const.tile([P, P], cfg.tdt)
            nc.sync.dma_start(iota_sb[:], iota_d[:])
            iota32_sb = const.tile([P, P], f32)
            nc.sync.dma_start(iota32_sb[:], iota32_d[:])
            ones_sb = const.tile([1, P], f32)
            nc.vector.memset(ones_sb[:], 1.0)

            ht = htp.tile([P, cfg.RPAD], f32)

            def dense(lhs_tiles, rhs_sb, bias_sb, F, l, odt):
                """y[m-block] = lhsT.T @ rhs + bias -> ag_in[l] (col-half split)."""
                for m in range(cfg.NBLK):
                    ps = psum.tile([P, F], f32, tag="ps")
                    for k, lt in enumerate(lhs_tiles):
                        nc.tensor.matmul(
                            out=ps[:], lhsT=lt[:, m * P:(m + 1) * P], rhs=rhs_sb[k][:],
                            start=(k == 0), stop=False)
                    nc.tensor.matmul(out=ps[:], lhsT=ones_sb[:], rhs=bias_sb[:],
                                     start=False, stop=True)
                    ysb = yp.tile([P, F], odt, tag="ysb")
                    nc.vector.tensor_copy(out=ysb[:], in_=ps[:])
                    r0 = m * P
                    r1 = min(cfg.NSH, r0 + P)
                    for h in (0, 1):
                        lo = max(r0, h * cfg.HALF)
                        hi = min(r1, (h + 1) * cfg.HALF)
                        if lo < hi:
                            nc.sync.dma_start(
                                out=ag_in[l][lo - h * cfg.HALF:hi - h * cfg.HALF,
                                             h * F:(h + 1) * F],
                                in_=ysb[lo - r0:hi - r0, :])

            def allgather(l):
                if no_cc:
                    # timeline-profiling stand-in: local copy, same deps
                    nc.sync.dma_start(out=tables[l][0:cfg.HALF, :],
                                      in_=ag_in[l][:])
                    return
                nc.gpsimd.collective_compute(
                    "AllGather", mybir.AluOpType.bypass,
                    replica_groups=[list(range(NCORES))],
                    ins=[ag_in[l][:]], outs=[tables[l][:]])

            def spmm(l):
                """tables[l] -> block outputs; updates ht (l<=L) or out (final)."""
                final = l == L + 1
                F = fdims[l]
                sdt = tdts[l]
                io_t = iota32_sb if final else iota_sb
                v_d, r_d = (val32_d, row32_d) if final else (val_d, row_d)
                mdt = f32 if final else cfg.tdt
                for s in range(cfg.NSB):
                    g = []
                    vt = []
                    rt = []
                    for h in (0, 1):
                        it = meta.tile([P, NIDX // 16], mybir.dt.int16,
                                       tag=f"it{h}")
                        nc.sync.dma_start(it[:], idx_d[h, s])
                        v = meta.tile([P, NPT], mdt, tag=f"vt{h}")
                        nc.sync.dma_start(v[:], v_d[h, s])
                        rr = meta.tile([P, NPT], mdt, tag=f"rt{h}")
                        nc.sync.dma_start(rr[:], r_d[h, s])
                        gt = gpool.tile([P, NPT, F], sdt, tag=f"g{h}")
                        # chunk calls to <=56 descs/lane: single_packet=True
                        # coalesces each lane's stream into ONE packet and the
                        # HW packet ceiling is 64 descriptors
                        CH = 7
                        for c0 in range(0, NPT, CH):
                            c1 = min(NPT, c0 + CH)
                            nc.gpsimd.dma_gather(
                                gt[:, c0:c1, :],
                                tables[l][:, h * F:(h + 1) * F],
                                it[:, c0 * 8:c1 * 8],
                                (c1 - c0) * P, (c1 - c0) * P, F,
                                elem_step=2 * F, single_packet=True)
                        g.append(gt)
                        vt.append(v)
                        rt.append(rr)
                    for bb in range(cfg.SBB):
                        b = s * cfg.SBB + bb
                        if final:
                            ps = psum.tile([P, F], f32, tag="ps")
                        else:
                            ps = psum.tile([F, P], f32, tag="ps")
                        k = 0
                        for h in (0, 1):
                            for t in range(TPB):
                                j = bb * TPB + t
                                sel = selp.tile([P, P], mdt, tag="sel")
                                nc.vector.scalar_tensor_tensor(
                                    out=sel[:], in0=io_t[:],
                                    scalar=rt[h][:, j:j + 1],
                                    in1=vt[h][:, j:j + 1].to_broadcast([P, P]),
                                    op0=mybir.AluOpType.is_equal,
                                    op1=mybir.AluOpType.mult)
                                if final:
                                    nc.tensor.matmul(
                                        out=ps[:], lhsT=sel[:], rhs=g[h][:, j, :],
                                        start=(k == 0), stop=(k == 2 * TPB - 1))
                                else:
                                    nc.tensor.matmul(
                                        out=ps[:], lhsT=g[h][:, j, :], rhs=sel[:],
                                        start=(k == 0), stop=(k == 2 * TPB - 1))
                                k += 1
                        if final:
                            osb = yp.tile([P, F], f32, tag="osb")
                            nc.vector.tensor_copy(out=osb[:], in_=ps[:])
                            r0 = b * P
                            r1 = min(cfg.NSH, r0 + P)
                            if r0 < r1:
                                nc.sync.dma_start(out=out_d[r0:r1, :],
                                                  in_=osb[: r1 - r0, :])
                        elif l == 0:
                            nc.vector.tensor_scalar(
                                out=ht[:, b * P:(b + 1) * P], in0=ps[:],
                                scalar1=0.0, scalar2=None,
                                op0=mybir.AluOpType.max)
                        else:
                            tmp = yp.tile([P, P], f32, tag="tmp")
                            nc.vector.tensor_scalar(
                                out=tmp[:], in0=ps[:],
                                scalar1=0.0, scalar2=dt_val,
                                op0=mybir.AluOpType.max,
                                op1=mybir.AluOpType.mult)
                            nc.vector.tensor_add(
                                out=ht[:, b * P:(b + 1) * P],
                                in0=ht[:, b * P:(b + 1) * P], in1=tmp[:])

            # ---- layer 1 dense (from xT in DRAM)
            with tc.tile_pool(name="xt", bufs=1) as xtp:
                xt_sb = xtp.tile([P, nkt * cfg.RPAD], f32)
                for k in range(nkt):
                    nc.sync.dma_start(
                        xt_sb[:, k * cfg.RPAD:(k + 1) * cfg.RPAD],
                        xT_d[k * P:(k + 1) * P, :])
                dense([xt_sb[:, k * cfg.RPAD:(k + 1) * cfg.RPAD]
                       for k in range(nkt)],
                      w1_sb, b1_sb, H, 0, tdts[0])
            allgather(0)
            spmm(0)
            for i in range(L):
                dense([ht], [wm_sb[i]], bm_sb[i], H, i + 1, tdts[i + 1])
                allgather(i + 1)
                spmm(i + 1)
            dense([ht], [w2_sb], b2_sb, C, L + 1, tdts[L + 1])
            allgather(L + 1)
            spmm(L + 1)

    nc.compile()
    return nc


# ------------------------------------------------------------------ driver

_CACHE = {}


def _get_program(cfg, TPB, dt_val):
    key = (cfg.N, cfg.E, cfg.tbl_fp16, TPB, float(dt_val))
    if key not in _CACHE:
        _CACHE[key] = build_program(cfg, TPB, dt_val)
    return _CACHE[key]


def prepare(cfg, inputs):
    """Preprocess inputs and build (cached) program. Returns (nc, in_maps)."""
    x = np.asarray(inputs["x"], np.float32)
    per_core, TPB = preprocess(cfg, x, inputs["edge_row"], inputs["edge_col"],
                               inputs["edge_val"])
    dt_val = float(np.asarray(inputs["time_step"]))
    nc = _get_program(cfg, TPB, dt_val)

    iota32 = np.tile(np.arange(P, dtype=np.float32), (P, 1))
    shared = dict(
        w1=np.asarray(inputs["w1"], np.float32),
        b1=np.asarray(inputs["b1"], np.float32).reshape(1, cfg.H),
        wm=np.asarray(inputs["wm"], np.float32),
        bm=np.asarray(inputs["bm"], np.float32).reshape(cfg.L, 1, cfg.H),
        w2=np.asarray(inputs["w2"], np.float32),
        b2=np.asarray(inputs["b2"], np.float32).reshape(1, cfg.C),
        iota=iota32.astype(cfg.tnp),
        iota32=iota32,
    )
    in_maps = [{**shared, **pc} for pc in per_core]
    return nc, in_maps


def run(cfg, inputs):
    nc, in_maps = prepare(cfg, inputs)
    res = run_bass_kernel_spmd(nc, in_maps, list(range(NCORES)))
    out = np.concatenate([res.results[r]["out"] for r in range(NCORES)], axis=0)
    return out.astype(np.float32)


def kernel(**inputs) -> np.ndarray:
    return run(CFG_FULL, inputs)


# ---------------------------------------------------- timing helper (test use)


def make_timed_runner(nc, in_maps):
    """Build a reusable jitted runner (no donation, device-resident operands).

    Mirrors bass2jax.run_bass_via_pjrt's multi-core path but keeps the jitted
    callable and device arrays so repeated calls measure dispatch+exec only.
    Returns (call_fn, out_unpack_fn).
    """
    import jax
    from jax.sharding import Mesh, PartitionSpec
    from jax.experimental.shard_map import shard_map
    from concourse import bass2jax
    from concourse.bass2jax import _bass_exec_p, partition_id_tensor

    bass2jax.install_neuronx_cc_hook()
    n_cores = len(in_maps)
    partition_name = nc.partition_id_tensor.name if nc.partition_id_tensor else None
    in_names, out_names, out_avals, zero_outs = [], [], [], []
    for alloc in nc.m.functions[0].allocations:
        if not isinstance(alloc, mybir.MemoryLocationSet):
            continue
        name = alloc.memorylocations[0].name
        if alloc.kind == "ExternalInput":
            if name != partition_name:
                in_names.append(name)
        elif alloc.kind == "ExternalOutput":
            out_names.append(name)
            out_avals.append(jax.core.ShapedArray(
                tuple(alloc.tensor_shape), mybir.dt.np(alloc.dtype)))
            zero_outs.append(np.zeros(tuple(alloc.tensor_shape),
                                      mybir.dt.np(alloc.dtype)))
    n_params = len(in_names)
    all_names = in_names + out_names
    if partition_name is not None:
        all_names.append(partition_name)

    def _body(*args):
        operands = list(args)
        if partition_name is not None:
            operands.append(partition_id_tensor())
        return tuple(_bass_exec_p.bind(
            *operands,
            out_avals=tuple(out_avals),
            in_names=tuple(all_names),
            out_names=tuple(out_names),
            lowering_input_output_aliases=(),
            sim_require_finite=True,
            sim_require_nnan=True,
            nc=nc,
        ))

    devices = jax.devices()[:n_cores]
    mesh = Mesh(np.asarray(devices), ("core",))
    spec_in = (PartitionSpec("core"),) * (n_params + len(out_names))
    spec_out = (PartitionSpec("core"),) * len(out_names)
    fn = jax.jit(shard_map(_body, mesh=mesh, in_specs=spec_in,
                           out_specs=spec_out, check_rep=False),
                 keep_unused=True)

    sharding = jax.sharding.NamedSharding(mesh, PartitionSpec("core"))
    dev_args = []
    for i, name in enumerate(in_names):
        cat = np.concatenate([np.asarray(m[name]) for m in in_maps], axis=0)
        dev_args.append(jax.device_put(cat, sharding))
    for z in zero_outs:
        cat = np.zeros((n_cores * z.shape[0], *z.shape[1:]), z.dtype)
        dev_args.append(jax.device_put(cat, sharding))

    def call():
        outs = fn(*dev_args)
        jax.block_until_ready(outs)
        return outs

    def unpack(outs):
        return [
            {name: np.asarray(outs[i]).reshape(n_cores, *out_avals[i].shape)[c]
             for i, name in enumerate(out_names)}
            for c in range(n_cores)
        ]

    return call, unpack
